# revision 2
# baseline (speedup 1.0000x reference)
"""Multi-head attention on 8 TRN2 NeuronCores — v2.

Sharding: core c handles batch b = c // 2 and head-half hh = c % 2
(8 of 16 heads). Each core computes a partial out^T; the host sums the
two partials per batch, adds bias, transposes back.

v2 changes over the baseline (fp8/Schraudolph variants were tested and
rejected: attention output is a near-cancelling weighted sum, so P/V
element noise passes through unattenuated -> fp8 gives ~4% rel err):
- softmax normalization: reciprocal row + two K=1 ones-matmuls
  broadcast it across partitions in PSUM; one tensor_tensor applies it
  (replaces memset + stream_shuffle + quadrant DMAs + 2 muls).
- V projection is pipelined into the first attention chunk's score/exp
  stream instead of running as a PE-only prologue.
- out-projection slices are interleaved into the last pair's chunks.
- all exp on ScalarE (the hard floor: 256 [128,1024] tiles); all
  psum->sbuf copies on VectorE.
- inputs arrive as 5 consolidated DMAs (x double-buffered) and output
  DMAs issue from the GPSIMD/SWDGE ring, so the SyncE HWDGE FIFO never
  backs up across loop iterations.
- separate PSUM pools: scores (2 bufs) / proj+out+bcast (1) / ctx (1),
  so the score->exp pipeline never blocks on a projection copy.
"""

import numpy as np
import ml_dtypes

import concourse.bacc as bacc
import concourse.tile as tile
import concourse.mybir as mybir
from concourse.bass_utils import run_bass_kernel_spmd

bf16 = ml_dtypes.bfloat16
FP32 = mybir.dt.float32
BF16 = mybir.dt.bfloat16
FP8 = mybir.dt.float8e4
FP8E5 = mybir.dt.float8e5
I8 = mybir.dt.int8
EXP = mybir.ActivationFunctionType.Exp
COPY = mybir.ActivationFunctionType.Copy
DRMODE = mybir.MatmulPerfMode.DoubleRow
MULT = mybir.AluOpType.mult
ADD = mybir.AluOpType.add

B, S, DL = 4, 2048, 1024
H, DH = 16, 64          # global heads
NH = 8                  # heads per core
HD = NH * DH            # 512 feature cols per core
NPAIR = NH // 2         # 4 head pairs
KT = DL // 128          # 8 k-tiles over d_latent
ST = S // 128           # 16 tiles over sequence
NQC = 4                 # q-chunks of 512 per head
SCALE = 1.0 / np.sqrt(DH)

N_CORES = 8


def _build(reps: int = 1, loop: int = 0, ablate=()):
    nc = bacc.Bacc(None, target_bir_lowering=False)

    xT = nc.dram_tensor("xT", [DL, S], BF16, kind="ExternalInput")
    wq = nc.dram_tensor("wq", [DL, HD], BF16, kind="ExternalInput")
    wk = nc.dram_tensor("wk", [DL, HD], BF16, kind="ExternalInput")
    wv = nc.dram_tensor("wv", [DL, HD], BF16, kind="ExternalInput")
    wo = nc.dram_tensor("wo", [HD, DL], BF16, kind="ExternalInput")
    out = nc.dram_tensor("out", [DL, S], FP32, kind="ExternalOutput")

    with tile.TileContext(nc) as tc:
        with (
            tc.tile_pool(name="persist", bufs=1) as pp,
            tc.tile_pool(name="dbl", bufs=2) as ppd,
            tc.tile_pool(name="pt", bufs=3) as ptp,
            tc.tile_pool(name="small", bufs=3) as smp,
            tc.tile_pool(name="outsb", bufs=2) as osp,
            tc.tile_pool(name="psA", bufs=2, space="PSUM") as psA,
            tc.tile_pool(name="psB", bufs=1, space="PSUM") as psB,
        ):
            if loop:
                with tc.For_i(0, loop, 1):
                    _body(nc, tc, pp, ppd, ptp, smp, osp, psA, psB,
                          xT, wq, wk, wv, wo, out, ablate)
            else:
                for _ in range(reps):
                    _body(nc, tc, pp, ppd, ptp, smp, osp, psA, psB,
                          xT, wq, wk, wv, wo, out, ablate)
    nc.compile()
    return nc


def _body(nc, tc, pp, ppd, ptp, smp, osp, psA, psB,
          xT, wq, wk, wv, wo, out, ablate=()):
    # ---- persistent tiles; x and weights loaded as single big DMAs ----
    xta = ppd.tile([128, KT, S], BF16, tag="xta", name="xta")
    wqa = pp.tile([128, KT, HD], BF16, tag="wqa", name="wqa")
    wka = pp.tile([128, KT, HD], BF16, tag="wka", name="wka")
    wva = pp.tile([128, KT, HD], BF16, tag="wva", name="wva")
    woa = pp.tile([128, NPAIR, DL], BF16, tag="woa", name="woa")
    xt = [xta[:, k, :] for k in range(KT)]
    wq_sb = [wqa[:, k, :] for k in range(KT)]
    wk_sb = [wka[:, k, :] for k in range(KT)]
    wv_sb = [wva[:, k, :] for k in range(KT)]
    wo_sb = [woa[:, t, :] for t in range(NPAIR)]
    qt = [pp.tile([128, S], BF16, tag=f"qt{p}", name=f"qt{p}") for p in range(NPAIR)]
    kt_ = [pp.tile([128, S], BF16, tag=f"kt{p}", name=f"kt{p}") for p in range(NPAIR)]
    # V|ones bf16: [sk-part, head, 68]
    vt = [pp.tile([128, NH, DH + 4], BF16, tag=f"vt{m}", name=f"vt{m}")
          for m in range(ST)]
    ct = [pp.tile([128, S], BF16, tag=f"ct{t}", name=f"ct{t}") for t in range(NPAIR)]
    ones = pp.tile([1, 128], BF16, tag="ones", name="ones")

    # ---- input DMAs (one per tensor) ----
    nc.sync.dma_start(xta[:, :, :], xT[:, :].rearrange("(k p) s -> p k s", p=128))
    nc.sync.dma_start(wqa[:, :, :], wq[:, :].rearrange("(k p) n -> p k n", p=128))
    nc.sync.dma_start(wka[:, :, :], wk[:, :].rearrange("(k p) n -> p k n", p=128))
    nc.sync.dma_start(wva[:, :, :], wv[:, :].rearrange("(k p) n -> p k n", p=128))
    nc.sync.dma_start(woa[:, :, :], wo[:, :].rearrange("(t p) n -> p t n", p=128))
    nc.vector.memset(ones[:, :], 1.0)

    # ---- emit helpers ----
    def vproj_group(m, tag="sc"):
        """V projection for sequence tile m -> vt[m] (bf16, ones col)."""
        ps = (psA if tag == "sc" else psB).tile([128, 1024], FP32, tag=tag,
                                                name="vps")
        for k in range(KT):
            nc.tensor.matmul(ps[:, 0:HD],
                             xt[k][:, m * 128:(m + 1) * 128],
                             wv_sb[k][:, :],
                             start=(k == 0), stop=(k == KT - 1))
        dst = vt[m][:, :, :]
        nc.vector.tensor_copy(dst[:, :, 0:DH],
                              ps[:, 0:HD].rearrange("p (h c) -> p h c", h=NH))
        nc.vector.memset(dst[:, :, DH:DH + 4], 1.0)

    def proj_group(p_, dst, w_sb, nch, tag="sc"):
        """Q^T or K^T projection: dst cols [nch*1024, (nch+1)*1024)."""
        ps = (psA if tag == "sc" else psB).tile([128, 1024], FP32, tag=tag,
                                                name="proj")
        for half in range(2):
            nsl = slice(half * 512, half * 512 + 512)
            rsl = slice(nch * 1024 + half * 512, nch * 1024 + half * 512 + 512)
            for k in range(KT):
                nc.tensor.matmul(ps[:, nsl],
                                 w_sb[k][:, p_ * 128:(p_ + 1) * 128],
                                 xt[k][:, rsl],
                                 start=(k == 0), stop=(k == KT - 1))
        nc.vector.tensor_copy(dst[:, nch * 1024:(nch + 1) * 1024], ps[:, :])

    def proj_groups_for(p_):
        if 'proj' in ablate or p_ >= NPAIR:
            return []
        return [(p_, dst, w_sb, nch)
                for dst, w_sb in ((qt[p_], wq_sb), (kt_[p_], wk_sb))
                for nch in range(2)]

    def out_group(mt, nch, tag="sc"):
        """out^T rows [mt*128,(mt+1)*128), cols [nch*1024,(nch+1)*1024)."""
        msl = slice(mt * 128, (mt + 1) * 128)
        ob = osp.tile([128, 1024], FP32, tag="ob")
        ps = (psA if tag == "sc" else psB).tile([128, 1024], FP32, tag=tag,
                                                name="outp")
        for half in range(2):
            nsl = slice(half * 512, half * 512 + 512)
            rsl = slice(nch * 1024 + half * 512, nch * 1024 + half * 512 + 512)
            for t in range(NPAIR):
                nc.tensor.matmul(ps[:, nsl],
                                 wo_sb[t][:, msl],
                                 ct[t][:, rsl],
                                 start=(t == 0), stop=(t == NPAIR - 1))
        nc.vector.tensor_copy(ob[:, :], ps[:, :])
        nc.gpsimd.dma_start(out[msl, nch * 1024:(nch + 1) * 1024], ob[:, :])

    # deferred softmax-normalize: at chunk end the denominator reciprocal is
    # taken; the partition-broadcast (two K=1 matmuls) and the in-place mul
    # are emitted early in the NEXT chunk so their latency hides.
    norm_pend = []

    def flush_norm():
        while norm_pend:
            p_, qsl_, rsrc_ = norm_pend.pop(0)
            bc = psA.tile([128, 1024], FP32, tag="sc", name="bcast")
            nc.tensor.matmul(bc[0:64, 0:512], ones[0:1, 0:64],
                             rsrc_[0:1, 0:512], start=True, stop=True,
                             tile_position=(0, 0))
            nc.tensor.matmul(bc[64:128, 0:512], ones[0:1, 0:64],
                             rsrc_[0:1, 512:1024], start=True, stop=True,
                             tile_position=(0, 64))
            nc.vector.tensor_tensor(ct[p_][:, qsl_], ct[p_][:, qsl_],
                                    bc[:, 0:512], MULT)

    # ---- prologue: pair0 Q/K projection (V interleaves into chunk0) ----
    for g in proj_groups_for(0):
        proj_group(*g)

    for p in range(NPAIR):
        pending_proj = proj_groups_for(p + 1)
        for qch in range(0 if 'attn' in ablate else NQC):
            parity = (NQC * p + qch) % 2
            borrow = f"ctx{1 - parity}"
            qsl = slice(qch * 512, (qch + 1) * 512)
            ctxp = psB.tile([DH + 4, 1024], FP32, tag=f"ctx{parity}",
                            name="ctxp")
            pend = None
            for kj in range(ST):
                ksl = slice(kj * 128, (kj + 1) * 128)
                sc = psA.tile([128, 1024], FP32, tag="sc", name="sc")
                nc.tensor.matmul(sc[:, 0:512], kt_[p][0:64, ksl],
                                 qt[p][0:64, qsl], start=True, stop=True,
                                 tile_position=(0, 0))
                nc.tensor.matmul(sc[:, 512:1024], kt_[p][64:128, ksl],
                                 qt[p][64:128, qsl], start=True, stop=True,
                                 tile_position=(64, 0))
                pt = ptp.tile([128, 1024], BF16, tag="pt", name="pt")
                nc.scalar.activation(pt[:, :], sc[:, :], EXP, scale=SCALE)
                if p == 0 and qch == 0 and 'vproj' not in ablate:
                    vproj_group(kj)

                def emit_ctx(jj, ptj):
                    for hi in range(2):
                        lh = 2 * p + hi
                        nc.tensor.matmul(
                            ctxp[:, hi * 512:(hi + 1) * 512],
                            vt[jj][:, lh, :],
                            ptj[:, hi * 512:(hi + 1) * 512],
                            start=(jj == 0), stop=(jj == ST - 1))
                if 'ctx' not in ablate:
                    if pend is not None:
                        emit_ctx(*pend)
                    pend = (kj, pt)
                if kj == 1:
                    flush_norm()
                # interleave out-proj groups for q cols 0:1024 (needs qch0+1
                # of all pairs normalized — true once (3,1) flushed above)
                if p == NPAIR - 1 and qch == 2 and 4 <= kj < 10 and 'out' not in ablate:
                    out_group(kj - 4, 0)
                if p == NPAIR - 1 and qch == 3 and kj in (4, 6) and 'out' not in ablate:
                    out_group(6 + (kj - 4) // 2, 0)
            if 'ctx' not in ablate:
                emit_ctx(*pend)

            # ---- chunk end: reciprocal of denominator, raw ctx -> sbuf ----
            if 'ctx' not in ablate:
                rsrc = smp.tile([1, 1024], BF16, tag="rsrc")
                with nc.allow_low_precision(reason="softmax denom recip bf16"):
                    nc.vector.reciprocal(rsrc[0:1, :], ctxp[DH:DH + 1, :])
                for hi in range(2):
                    nc.vector.tensor_copy(ct[p][hi * 64:(hi + 1) * 64, qsl],
                                          ctxp[0:DH, hi * 512:(hi + 1) * 512])
                norm_pend.append((p, qsl, rsrc))
            if pending_proj:
                proj_group(*pending_proj.pop(0))

        for g in pending_proj:
            proj_group(*g)

    flush_norm()

    # ---- remaining out-projection ----
    if 'out' not in ablate:
        for mt in range(6, KT):
            out_group(mt, 0)
        for mt in range(KT):
            out_group(mt, 1)


_NC_CACHE = {}


def _get_nc(reps: int = 1):
    if reps not in _NC_CACHE:
        _NC_CACHE[reps] = _build(reps)
    return _NC_CACHE[reps]


def shard_inputs(x, w_q, w_kv, w_out):
    ins = []
    for c in range(N_CORES):
        b, hh = c // 2, c % 2
        fsl = slice(hh * HD, (hh + 1) * HD)
        ins.append({
            "xT": np.ascontiguousarray(x[b].T).astype(bf16),
            "wq": np.ascontiguousarray(w_q[:, fsl]).astype(bf16),
            "wk": np.ascontiguousarray(w_kv[:, fsl]).astype(bf16),
            "wv": np.ascontiguousarray(w_kv[:, H * DH:][:, fsl]).astype(bf16),
            "wo": np.ascontiguousarray(w_out[fsl, :]).astype(bf16),
        })
    return ins


def unshard_output(results, b_out):
    out = np.empty((B, S, DL), np.float32)
    for b in range(B):
        acc = results[2 * b]["out"] + results[2 * b + 1]["out"]   # [DL, S]
        out[b] = acc.T + b_out
    return out


def kernel(x, w_q, w_kv, w_out, b_out):
    nc = _get_nc()
    ins = shard_inputs(x, w_q, w_kv, w_out)
    res = run_bass_kernel_spmd(nc, ins, core_ids=list(range(N_CORES)))
    return unshard_output(res.results, b_out)
